# revision 10
# baseline (speedup 1.0000x reference)
"""LocalWindowAttention (block-causal) Trainium2 kernel, 8 NeuronCores.

Sharding: tensor-parallel over heads for QKV + attention, then an
AllToAll re-shards by tokens for the output projection:
  - core c owns head-columns [c*128, (c+1)*128) of D=1024 (2 heads):
    computes its Q/K/V projections (transposed layout) and block-causal
    attention for its 2 heads over all 2048 tokens,
  - the normalized pre-Wo activations are AllToAll'd: core c sends its
    [128, 256]-token slices to every core and receives all 16 heads'
    activations for ITS 256 tokens (512KB per core on the wire vs 4MB
    for an AllGather),
  - core c applies the FULL Wo to produce y^T[:, c*256:(c+1)*256];
    the host concatenates token slices.

v3 notes (173us -> target <100us):
  - one AllToAll at the end instead of per-chunk AllGathers: the ncfw
    collective path has a ~60us warmup before the first op can complete
    and each op costs 10-30us, so exactly one small collective whose
    input is ready right when attention ends is optimal.
  - bf16 datapath except PSUM accumulation and the final output.
  - fully chunk-pipelined: x streams per 512-column chunk; chunk t+1's
    projections interleave with chunk t's attention; PE stays dense so
    the HAM clock stays at 2.4 GHz.
  - reciprocal_approx_fast for softmax denominators (DVE RECIPROCAL is
    ~8 cyc/elem, 3.3us per row; the approx op is ~5x faster).

Attention runs in S^T layout (keys on partitions, queries free):
S^T tile = K_chunk @ Q^T. No max-subtraction needed (scores bounded).
The two heads' score matmuls use contraction rows 0-63 / 64-127 and
different PSUM banks, so the PE runs them concurrently (row-group
tiling); one ACT instruction computes both heads' exp. V is transposed
to natural layout per chunk with a trailing ones column per head so the
softmax denominator falls out of the attn@V matmul as row 64.
"""

import numpy as np
from ml_dtypes import bfloat16

import concourse.bacc as bacc
import concourse.tile as tile
from concourse import mybir
from concourse.bass_utils import run_bass_kernel_spmd
from concourse.masks import make_identity

B, T, D = 1, 2048, 1024
H, HD, W = 16, 64, 128
N_CORES = 8
HS = D // N_CORES        # 128 head-columns per core (2 heads)
HPC = H // N_CORES       # heads per core
QW = 512                 # query-chunk width (free dim of S^T tiles)
NQ = T // QW             # 4 query chunks
NK = T // W              # 16 key chunks of 128
ND = D // 128            # 8 contraction chunks over D
TS = T // N_CORES        # 256-token output slice per core
SCALE = HD ** -0.5

F32 = mybir.dt.float32
BF16 = mybir.dt.bfloat16
Exp = mybir.ActivationFunctionType.Exp

_compiled = {}


def _build():
    nc = bacc.Bacc("TRN2", target_bir_lowering=False, debug=False,
                   num_devices=N_CORES)
    xT_ap = nc.dram_tensor("xT", [D, T], BF16, kind="ExternalInput").ap()
    wq_ap = nc.dram_tensor("wq", [D, HS], BF16, kind="ExternalInput").ap()
    wk_ap = nc.dram_tensor("wk", [D, HS], BF16, kind="ExternalInput").ap()
    wv_ap = nc.dram_tensor("wv", [D, HS], BF16, kind="ExternalInput").ap()
    wo_ap = nc.dram_tensor("wo", [D, D], BF16, kind="ExternalInput").ap()
    y_ap = nc.dram_tensor("y", [D, TS], F32, kind="ExternalOutput").ap()

    with tile.TileContext(nc) as tc:
        _body(tc, xT_ap, wq_ap, wk_ap, wv_ap, wo_ap, y_ap)
    nc.compile()
    return nc


def _body(tc, xT_ap, wq_ap, wk_ap, wv_ap, wo_ap, y_ap):
    nc = tc.nc
    from contextlib import ExitStack
    with ExitStack() as ctx:
        singles = ctx.enter_context(tc.tile_pool(name="singles", bufs=1))
        work = ctx.enter_context(tc.tile_pool(name="work", bufs=3))
        es_pool = ctx.enter_context(tc.tile_pool(name="es_pool", bufs=6))
        vt_pool = ctx.enter_context(tc.tile_pool(name="vt_pool", bufs=2))
        dram = ctx.enter_context(tc.tile_pool(name="dram", bufs=1, space="DRAM"))
        # PSUM budget (8 banks): scores 2x2 + attn@V accum 2 + aux 2
        pa = ctx.enter_context(tc.tile_pool(name="pa", bufs=2, space="PSUM"))
        po = ctx.enter_context(tc.tile_pool(name="po", bufs=1, space="PSUM"))
        paux = ctx.enter_context(tc.tile_pool(name="paux", bufs=2, space="PSUM"))

        # ---- input DMAs, ordered so chunk-0 projections start ASAP ----
        wq = singles.tile([128, ND, HS], BF16, tag="wq")
        wk = singles.tile([128, ND, HS], BF16, tag="wk")
        wv = singles.tile([128, ND, HS], BF16, tag="wv")
        wo = singles.tile([128, ND, D], BF16, tag="wo")
        xcs = [singles.tile([128, ND, QW], BF16, tag=f"x{t}", name=f"xc{t}")
               for t in range(NQ)]
        x_r = xT_ap.rearrange("(c p) (t m) -> p c t m", p=128, t=NQ)
        nc.sync.dma_start(out=wq[:], in_=wq_ap.rearrange("(c p) m -> p c m", p=128))
        nc.sync.dma_start(out=xcs[0][:], in_=x_r[:, :, 0, :])
        nc.sync.dma_start(out=wk[:], in_=wk_ap.rearrange("(c p) m -> p c m", p=128))
        nc.sync.dma_start(out=wv[:], in_=wv_ap.rearrange("(c p) m -> p c m", p=128))
        for t in range(1, NQ):
            nc.sync.dma_start(out=xcs[t][:], in_=x_r[:, :, t, :])
        # full Wo (only consumed after the AllToAll, ~70us in)
        nc.scalar.dma_start(out=wo[:], in_=wo_ap.rearrange("(c p) m -> p c m", p=128))

        ident_f32 = singles.tile([128, 128], F32, tag="ident_f32")
        make_identity(nc, ident_f32)
        ident = singles.tile([128, 128], BF16, tag="ident")
        nc.vector.tensor_copy(ident[:], ident_f32[:])

        qT = singles.tile([128, T], BF16, tag="qT")
        kT = singles.tile([128, T], BF16, tag="kT")
        # V natural layout: [key 128, NK, hd0|1|hd1|1]; head h's stationary
        # operand is vn[:, tk, 65h:65h+65] = [hd, ones] so row 64 of the
        # attn@V output is the softmax denominator.
        vn = singles.tile([128, NK, 2 * (HD + 1)], BF16, tag="vn")
        nc.vector.memset(vn[:, :, HD], 1.0)
        nc.vector.memset(vn[:, :, 2 * HD + 1], 1.0)
        outT = singles.tile([128, T], BF16, tag="outT")

        def proj(t):
            cols = slice(t * QW, (t + 1) * QW)
            for dst, w in ((qT, wq), (kT, wk), (None, wv)):
                ps = paux.tile([128, QW], F32, tag="aux", name=f"pj{t}")
                for d in range(ND):
                    nc.tensor.matmul(ps[:], w[:, d, :], xcs[t][:, d, :],
                                     start=(d == 0), stop=(d == ND - 1))
                if dst is not None:
                    nc.vector.tensor_copy(dst[:, cols], ps[:])
                else:
                    vt = vt_pool.tile([128, QW], BF16, tag="vt", name=f"vt{t}")
                    nc.vector.tensor_copy(vt[:], ps[:])
                    for j in range(4):
                        tk = 4 * t + j
                        ps_t = paux.tile([128, 128], BF16, tag="aux",
                                         name=f"tr{tk}")
                        nc.tensor.transpose(ps_t[:], vt[:, j * W:(j + 1) * W],
                                            ident[:])
                        src = ps_t[:].rearrange("p (h m) -> p h m", h=2)
                        dst3 = vn[:, tk, :].rearrange("p (h m) -> p h m", h=2)
                        nc.vector.tensor_copy(dst3[:, :, 0:HD], src[:])

        # AllToAll buffers: shard j of ain (outT token-slice j) goes to core
        # j; block i of aout is core i's head-slice for MY 256 tokens.
        ain = dram.tile([N_CORES, HS, TS], BF16, name="ain")
        aout = dram.tile([N_CORES, HS, TS], BF16, name="aout")

        def attention(t):
            cols = slice(t * QW, (t + 1) * QW)
            n_tk = 4 * t + 4
            ps_o = [po.tile([HD + 1, QW], F32, tag=f"o{h}", name=f"ps_o{h}")
                    for h in range(HPC)]

            def s_exp(tk):
                qs = max(0, (tk - 4 * t) * W)  # masked cols before qs
                # both heads' scores in one double-wide PSUM tile: the two
                # matmuls use contraction rows 0-63 / 64-127 -> different
                # PE row groups -> concurrent; one ACT instruction exps both
                ps_s = pa.tile([128, 2 * QW], F32, tag="s", name="ps_s")
                for h in range(HPC):
                    hrows = slice(h * HD, (h + 1) * HD)
                    nc.tensor.matmul(
                        ps_s[:, h * QW + qs:(h + 1) * QW],
                        kT[hrows, tk * W:(tk + 1) * W],
                        qT[hrows, t * QW + qs:(t + 1) * QW],
                        start=True, stop=True)
                e = es_pool.tile([128, 2 * QW], BF16, tag="es", name="es")
                nc.scalar.activation(out=e[:, qs:], in_=ps_s[:, qs:],
                                     func=Exp, scale=SCALE)
                return e

            def av(tk, e):
                qs = max(0, (tk - 4 * t) * W)
                for h in range(HPC):
                    nc.tensor.matmul(ps_o[h][:, qs:],
                                     vn[:, tk, h * (HD + 1):(h + 1) * (HD + 1)],
                                     e[:, h * QW + qs:(h + 1) * QW],
                                     start=(tk == 0), stop=(tk == n_tk - 1))

            # scores/exp run two key-chunks ahead of the attn@V accumulation
            pend = [s_exp(0), s_exp(1)]
            for tk in range(2, n_tk):
                pend.append(s_exp(tk))
                av(tk - 2, pend.pop(0))
            av(n_tk - 2, pend.pop(0))
            av(n_tk - 1, pend.pop(0))

            # normalize: rows 0..63 of each ps_o scaled by 1/row64
            for h in range(HPC):
                den = work.tile([1, QW], F32, tag=f"den{h}", name=f"den{t}_{h}")
                nc.vector.tensor_copy(den[:], ps_o[h][HD:HD + 1, :])
                rec = work.tile([1, QW], F32, tag=f"rec{h}", name=f"rec{t}_{h}")
                nc.vector.reciprocal_approx_fast(rec[:], den[:])
                bc = work.tile([HD, QW], F32, tag=f"bc{h}", name=f"bc{t}_{h}")
                nc.gpsimd.partition_broadcast(bc[:], rec[:], channels=HD)
                nc.vector.tensor_mul(outT[h * HD:(h + 1) * HD, cols],
                                     ps_o[h][0:HD, :], bc[:])

            # ship this chunk's two token-slices into the AllToAll input
            for j in range(2):
                c0 = t * QW + j * TS
                nc.sync.dma_start(out=ain[2 * t + j],
                                  in_=outT[:, c0:c0 + TS])

        proj(0)
        for t in range(NQ):
            attention(t)
            if t < NQ - 1:
                proj(t + 1)

        nc.gpsimd.collective_compute(
            "AllToAll", mybir.AluOpType.bypass,
            replica_groups=[list(range(N_CORES))],
            ins=[ain.opt()], outs=[aout.opt()])

        # output projection for my 256 tokens with the full Wo:
        # y^T[r-block, :] = sum_c Wo[c-block, r-block]^T @ G_c
        g2 = singles.tile([128, N_CORES, TS], BF16, tag="g2")
        nc.sync.dma_start(out=g2[:], in_=aout.rearrange("c p m -> p c m"))
        ySB = singles.tile([128, ND, TS], F32, tag="ySB")
        for r in range(ND):
            ps_y = paux.tile([128, TS], F32, tag="aux", name=f"py{r}")
            for c in range(N_CORES):
                nc.tensor.matmul(ps_y[:], wo[:, c, r * 128:(r + 1) * 128],
                                 g2[:, c, :],
                                 start=(c == 0), stop=(c == N_CORES - 1))
            if r % 2 == 0:
                nc.vector.tensor_copy(ySB[:, r, :], ps_y[:])
            else:
                nc.scalar.copy(ySB[:, r, :], ps_y[:])
        nc.sync.dma_start(out=y_ap.rearrange("(r p) m -> p r m", p=128),
                          in_=ySB[:])


def _make_in_maps(x, Wq, Wk, Wv, Wo):
    xT = np.asarray(x, np.float32).reshape(T, D).T
    xT = np.ascontiguousarray(xT).astype(bfloat16)
    wo_full = np.ascontiguousarray(np.asarray(Wo, np.float32)).astype(bfloat16)
    in_maps = []
    for c in range(N_CORES):
        hs = slice(c * HS, (c + 1) * HS)
        in_maps.append({
            "xT": xT,
            "wq": np.ascontiguousarray(np.asarray(Wq, np.float32)[:, hs]).astype(bfloat16),
            "wk": np.ascontiguousarray(np.asarray(Wk, np.float32)[:, hs]).astype(bfloat16),
            "wv": np.ascontiguousarray(np.asarray(Wv, np.float32)[:, hs]).astype(bfloat16),
            "wo": wo_full,
        })
    return in_maps


def kernel(x, Wq, Wk, Wv, Wo):
    if "nc" not in _compiled:
        _compiled["nc"] = _build()
    nc = _compiled["nc"]
    in_maps = _make_in_maps(x, Wq, Wk, Wv, Wo)
    res = run_bass_kernel_spmd(nc, in_maps, list(range(N_CORES)))
    finalT = np.concatenate([res.results[c]["y"] for c in range(N_CORES)], axis=1)
    return np.ascontiguousarray(finalT.T).reshape(B, T, D).astype(np.float32)


# revision 11
# speedup vs baseline: 1.0457x; 1.0457x over previous
"""LocalWindowAttention (block-causal) Trainium2 kernel, 8 NeuronCores.

Sharding: tensor-parallel over heads. Core c owns head-columns
[c*128, (c+1)*128) of the D=1024 hidden dim (2 heads x head_dim 64):
  - computes Q/K/V projections for its head slice (transposed layout),
  - block-causal attention for its 2 heads,
  - ONE AllGather of the normalized pre-Wo activations [128, 2048]
    bf16 (the collective runtime here has a ~70us warmup and ~25-40us
    per-op cost, so exactly one op whose input is ready right when
    attention ends is optimal; AllToAll measured WORSE despite moving
    8x less data - per-pair shard overheads dominate),
  - partial output projection with its 128 columns of Wo over the
    gathered activations; core c keeps rows [c*128,(c+1)*128) of
    final^T [1024, 2048]. Host reassembles.

Perf structure (baseline 215us -> v2 173us -> this):
  - bf16 datapath except PSUM accumulation and the final output.
  - fully chunk-pipelined: x streams per 512-column chunk in half-chunk
    DMAs; chunk t+1's projections interleave with chunk t's attention;
    PE starts ~7us in and stays dense (HAM stays at 2.4 GHz).
  - the 2MB full-Wo load is issued AFTER the attention emission so its
    packets don't steal DMA bandwidth from the critical x0 load.
  - reciprocal_approx_fast for softmax denominators (DVE RECIPROCAL is
    ~8 cyc/elem; the approx op is ~5x faster, 18-bit accurate).

Attention runs in S^T layout (keys on partitions, queries free):
S^T tile = K_chunk @ Q^T. No max-subtraction needed (scores bounded).
The two heads' score matmuls use contraction rows 0-63 / 64-127 and
different PSUM banks, so the PE runs them concurrently (row-group
tiling); one ACT instruction computes both heads' exp. V is transposed
to natural layout per chunk with a trailing ones column per head so the
softmax denominator falls out of the attn@V matmul as row 64.
"""

import numpy as np
from ml_dtypes import bfloat16

import concourse.bacc as bacc
import concourse.tile as tile
from concourse import mybir
from concourse.bass_utils import run_bass_kernel_spmd
from concourse.masks import make_identity

B, T, D = 1, 2048, 1024
H, HD, W = 16, 64, 128
N_CORES = 8
HS = D // N_CORES        # 128 head-columns per core (2 heads)
HPC = H // N_CORES       # heads per core
QW = 512                 # query-chunk width (free dim of S^T tiles)
NQ = T // QW             # 4 query chunks
NK = T // W              # 16 key chunks of 128
ND = D // 128            # 8 contraction chunks over D
SCALE = HD ** -0.5

F32 = mybir.dt.float32
BF16 = mybir.dt.bfloat16
Exp = mybir.ActivationFunctionType.Exp

_compiled = {}


def _build():
    nc = bacc.Bacc("TRN2", target_bir_lowering=False, debug=False,
                   num_devices=N_CORES)
    xT_ap = nc.dram_tensor("xT", [D, T], BF16, kind="ExternalInput").ap()
    wq_ap = nc.dram_tensor("wq", [D, HS], BF16, kind="ExternalInput").ap()
    wk_ap = nc.dram_tensor("wk", [D, HS], BF16, kind="ExternalInput").ap()
    wv_ap = nc.dram_tensor("wv", [D, HS], BF16, kind="ExternalInput").ap()
    wo_ap = nc.dram_tensor("wo", [D, HS], BF16, kind="ExternalInput").ap()
    y_ap = nc.dram_tensor("y", [HS, T], F32, kind="ExternalOutput").ap()

    with tile.TileContext(nc) as tc:
        _body(tc, xT_ap, wq_ap, wk_ap, wv_ap, wo_ap, y_ap)
    nc.compile()
    return nc


def _body(tc, xT_ap, wq_ap, wk_ap, wv_ap, wo_ap, y_ap):
    nc = tc.nc
    from contextlib import ExitStack
    with ExitStack() as ctx:
        singles = ctx.enter_context(tc.tile_pool(name="singles", bufs=1))
        work = ctx.enter_context(tc.tile_pool(name="work", bufs=3))
        es_pool = ctx.enter_context(tc.tile_pool(name="es_pool", bufs=6))
        vt_pool = ctx.enter_context(tc.tile_pool(name="vt_pool", bufs=2))
        g_pool = ctx.enter_context(tc.tile_pool(name="g_pool", bufs=2))
        dram = ctx.enter_context(tc.tile_pool(name="dram", bufs=1, space="DRAM"))
        # PSUM budget (8 banks): scores 2x2 + attn@V accum 2 + aux 2
        pa = ctx.enter_context(tc.tile_pool(name="pa", bufs=2, space="PSUM"))
        po = ctx.enter_context(tc.tile_pool(name="po", bufs=1, space="PSUM"))
        paux = ctx.enter_context(tc.tile_pool(name="paux", bufs=2, space="PSUM"))

        # ---- input DMAs, ordered so chunk-0 projections start ASAP ----
        # x chunks split in half-d so the first q-proj matmuls can start
        # after ~0.75MB instead of ~1.25MB.
        wq = singles.tile([128, ND, HS], BF16, tag="wq")
        wk = singles.tile([128, ND, HS], BF16, tag="wk")
        wv = singles.tile([128, ND, HS], BF16, tag="wv")
        wo = singles.tile([128, ND, HS], BF16, tag="wo")
        xcs = [singles.tile([128, ND, QW], BF16, tag=f"x{t}", name=f"xc{t}")
               for t in range(NQ)]
        x_r = xT_ap.rearrange("(c p) (t m) -> p c t m", p=128, t=NQ)
        nc.sync.dma_start(out=wq[:], in_=wq_ap.rearrange("(c p) m -> p c m", p=128))
        nc.sync.dma_start(out=xcs[0][:, 0:4, :], in_=x_r[:, 0:4, 0, :])
        nc.sync.dma_start(out=xcs[0][:, 4:8, :], in_=x_r[:, 4:8, 0, :])
        nc.sync.dma_start(out=wk[:], in_=wk_ap.rearrange("(c p) m -> p c m", p=128))
        nc.sync.dma_start(out=wv[:], in_=wv_ap.rearrange("(c p) m -> p c m", p=128))
        for t in range(1, NQ):
            nc.sync.dma_start(out=xcs[t][:, 0:4, :], in_=x_r[:, 0:4, t, :])
            nc.sync.dma_start(out=xcs[t][:, 4:8, :], in_=x_r[:, 4:8, t, :])

        ident_f32 = singles.tile([128, 128], F32, tag="ident_f32")
        make_identity(nc, ident_f32)
        ident = singles.tile([128, 128], BF16, tag="ident")
        nc.vector.tensor_copy(ident[:], ident_f32[:])

        qT = singles.tile([128, T], BF16, tag="qT")
        kT = singles.tile([128, T], BF16, tag="kT")
        # V natural layout: [key 128, NK, hd0|1|hd1|1]; head h's stationary
        # operand is vn[:, tk, 65h:65h+65] = [hd, ones] so row 64 of the
        # attn@V output is the softmax denominator.
        vn = singles.tile([128, NK, 2 * (HD + 1)], BF16, tag="vn")
        nc.vector.memset(vn[:, :, HD], 1.0)
        nc.vector.memset(vn[:, :, 2 * HD + 1], 1.0)
        outT = singles.tile([128, T], BF16, tag="outT")

        def proj(t):
            cols = slice(t * QW, (t + 1) * QW)
            for dst, w in ((qT, wq), (kT, wk), (None, wv)):
                ps = paux.tile([128, QW], F32, tag="aux", name=f"pj{t}")
                for d in range(ND):
                    nc.tensor.matmul(ps[:], w[:, d, :], xcs[t][:, d, :],
                                     start=(d == 0), stop=(d == ND - 1))
                if dst is not None:
                    nc.vector.tensor_copy(dst[:, cols], ps[:])
                else:
                    vt = vt_pool.tile([128, QW], BF16, tag="vt", name=f"vt{t}")
                    nc.vector.tensor_copy(vt[:], ps[:])
                    for j in range(4):
                        tk = 4 * t + j
                        ps_t = paux.tile([128, 128], BF16, tag="aux",
                                         name=f"tr{tk}")
                        nc.tensor.transpose(ps_t[:], vt[:, j * W:(j + 1) * W],
                                            ident[:])
                        src = ps_t[:].rearrange("p (h m) -> p h m", h=2)
                        dst3 = vn[:, tk, :].rearrange("p (h m) -> p h m", h=2)
                        nc.vector.tensor_copy(dst3[:, :, 0:HD], src[:])

        ag_in = dram.tile([HS, T], BF16, name="ag_in")
        ag_out = dram.tile([N_CORES, HS, T], BF16, addr_space="Shared",
                           name="ag_out")

        def attention(t):
            cols = slice(t * QW, (t + 1) * QW)
            n_tk = 4 * t + 4
            ps_o = [po.tile([HD + 1, QW], F32, tag=f"o{h}", name=f"ps_o{h}")
                    for h in range(HPC)]

            def s_exp(tk):
                qs = max(0, (tk - 4 * t) * W)  # masked cols before qs
                # both heads' scores in one double-wide PSUM tile: the two
                # matmuls use contraction rows 0-63 / 64-127 -> different
                # PE row groups -> concurrent; one ACT instruction exps both
                ps_s = pa.tile([128, 2 * QW], F32, tag="s", name="ps_s")
                for h in range(HPC):
                    hrows = slice(h * HD, (h + 1) * HD)
                    nc.tensor.matmul(
                        ps_s[:, h * QW + qs:(h + 1) * QW],
                        kT[hrows, tk * W:(tk + 1) * W],
                        qT[hrows, t * QW + qs:(t + 1) * QW],
                        start=True, stop=True)
                e = es_pool.tile([128, 2 * QW], BF16, tag="es", name="es")
                nc.scalar.activation(out=e[:, qs:], in_=ps_s[:, qs:],
                                     func=Exp, scale=SCALE)
                return e

            def av(tk, e):
                qs = max(0, (tk - 4 * t) * W)
                for h in range(HPC):
                    nc.tensor.matmul(ps_o[h][:, qs:],
                                     vn[:, tk, h * (HD + 1):(h + 1) * (HD + 1)],
                                     e[:, h * QW + qs:(h + 1) * QW],
                                     start=(tk == 0), stop=(tk == n_tk - 1))

            # scores/exp run two key-chunks ahead of the attn@V accumulation
            pend = [s_exp(0), s_exp(1)]
            for tk in range(2, n_tk):
                pend.append(s_exp(tk))
                av(tk - 2, pend.pop(0))
            av(n_tk - 2, pend.pop(0))
            av(n_tk - 1, pend.pop(0))

            # normalize: rows 0..63 of each ps_o scaled by 1/row64
            for h in range(HPC):
                den = work.tile([1, QW], F32, tag=f"den{h}", name=f"den{t}_{h}")
                nc.vector.tensor_copy(den[:], ps_o[h][HD:HD + 1, :])
                rec = work.tile([1, QW], F32, tag=f"rec{h}", name=f"rec{t}_{h}")
                nc.vector.reciprocal_approx_fast(rec[:], den[:])
                bc = work.tile([HD, QW], F32, tag=f"bc{h}", name=f"bc{t}_{h}")
                nc.gpsimd.partition_broadcast(bc[:], rec[:], channels=HD)
                nc.vector.tensor_mul(outT[h * HD:(h + 1) * HD, cols],
                                     ps_o[h][0:HD, :], bc[:])

            # stage this chunk into the AllGather input buffer
            nc.sync.dma_start(out=ag_in[:, cols], in_=outT[:, cols])

        proj(0)
        for t in range(NQ):
            attention(t)
            if t < NQ - 1:
                proj(t + 1)

        # Wo only feeds the post-AllGather matmuls (~100us in); issuing its
        # 256KB load here keeps the early DMA lanes clear for x chunks.
        nc.scalar.dma_start(out=wo[:], in_=wo_ap.rearrange("(c p) m -> p c m", p=128))

        nc.gpsimd.collective_compute(
            "AllGather", mybir.AluOpType.bypass,
            replica_groups=[list(range(N_CORES))],
            ins=[ag_in.opt()], outs=[ag_out.opt()])

        # output projection, per query chunk with pipelined gather reads
        g_r = ag_out.rearrange("c p (t m) -> p c t m", t=NQ)
        for t in range(NQ):
            cols = slice(t * QW, (t + 1) * QW)
            g = g_pool.tile([128, N_CORES, QW], BF16, tag="g", name=f"g{t}")
            nc.scalar.dma_start(out=g[:], in_=g_r[:, :, t, :])
            ps_y = paux.tile([128, QW], F32, tag="aux", name=f"py{t}")
            for c in range(N_CORES):
                nc.tensor.matmul(ps_y[:], wo[:, c, :], g[:, c, :],
                                 start=(c == 0), stop=(c == N_CORES - 1))
            cy = work.tile([128, QW], F32, tag="cy", name=f"cy{t}")
            if t % 2 == 0:
                nc.vector.tensor_copy(cy[:], ps_y[:])
            else:
                nc.scalar.copy(cy[:], ps_y[:])
            nc.sync.dma_start(out=y_ap[:, cols], in_=cy[:])


def _make_in_maps(x, Wq, Wk, Wv, Wo):
    xT = np.asarray(x, np.float32).reshape(T, D).T
    xT = np.ascontiguousarray(xT).astype(bfloat16)
    in_maps = []
    for c in range(N_CORES):
        hs = slice(c * HS, (c + 1) * HS)
        in_maps.append({
            "xT": xT,
            "wq": np.ascontiguousarray(np.asarray(Wq, np.float32)[:, hs]).astype(bfloat16),
            "wk": np.ascontiguousarray(np.asarray(Wk, np.float32)[:, hs]).astype(bfloat16),
            "wv": np.ascontiguousarray(np.asarray(Wv, np.float32)[:, hs]).astype(bfloat16),
            "wo": np.ascontiguousarray(np.asarray(Wo, np.float32)[:, hs]).astype(bfloat16),
        })
    return in_maps


def kernel(x, Wq, Wk, Wv, Wo):
    if "nc" not in _compiled:
        _compiled["nc"] = _build()
    nc = _compiled["nc"]
    in_maps = _make_in_maps(x, Wq, Wk, Wv, Wo)
    res = run_bass_kernel_spmd(nc, in_maps, list(range(N_CORES)))
    finalT = np.concatenate([res.results[c]["y"] for c in range(N_CORES)], axis=0)
    return np.ascontiguousarray(finalT.T).reshape(B, T, D).astype(np.float32)


# revision 14
# speedup vs baseline: 1.0641x; 1.0176x over previous
"""LocalWindowAttention (block-causal) Trainium2 kernel, 8 NeuronCores.

Sharding: tensor-parallel over heads. Core c owns head-columns
[c*128, (c+1)*128) of the D=1024 hidden dim (2 heads x head_dim 64):
  - computes Q/K/V projections for its head slice (transposed layout),
  - block-causal attention for its 2 heads,
  - ONE AllGather of the normalized pre-Wo activations [128, 2048]
    bf16 (the collective runtime here has a ~70us warmup and ~25-40us
    per-op cost, so exactly one op whose input is ready right when
    attention ends is optimal; AllToAll measured WORSE despite moving
    8x less data - per-pair shard overheads dominate),
  - partial output projection with its 128 columns of Wo over the
    gathered activations; core c keeps rows [c*128,(c+1)*128) of
    final^T [1024, 2048]. Host reassembles.

Perf structure (baseline 215us -> v2 173us -> this):
  - bf16 datapath except PSUM accumulation and the final output.
  - fully chunk-pipelined: x streams per 512-column chunk in half-chunk
    DMAs; chunk t+1's projections interleave with chunk t's attention;
    PE starts ~7us in and stays dense (HAM stays at 2.4 GHz).
  - the 2MB full-Wo load is issued AFTER the attention emission so its
    packets don't steal DMA bandwidth from the critical x0 load.
  - reciprocal_approx_fast for softmax denominators (DVE RECIPROCAL is
    ~8 cyc/elem; the approx op is ~5x faster, 18-bit accurate).

Attention runs in S^T layout (keys on partitions, queries free):
S^T tile = K_chunk @ Q^T. No max-subtraction needed (scores bounded).
The two heads' score matmuls use contraction rows 0-63 / 64-127 and
different PSUM banks, so the PE runs them concurrently (row-group
tiling); one ACT instruction computes both heads' exp. V is transposed
to natural layout per chunk with a trailing ones column per head so the
softmax denominator falls out of the attn@V matmul as row 64.
"""

import numpy as np
from ml_dtypes import bfloat16

import concourse.bacc as bacc
import concourse.tile as tile
from concourse import mybir
from concourse.bass_utils import run_bass_kernel_spmd
from concourse.masks import make_identity

B, T, D = 1, 2048, 1024
H, HD, W = 16, 64, 128
N_CORES = 8
HS = D // N_CORES        # 128 head-columns per core (2 heads)
HPC = H // N_CORES       # heads per core
QW = 512                 # query-chunk width (free dim of S^T tiles)
NQ = T // QW             # 4 query chunks
NK = T // W              # 16 key chunks of 128
ND = D // 128            # 8 contraction chunks over D
SCALE = HD ** -0.5

F32 = mybir.dt.float32
BF16 = mybir.dt.bfloat16
Exp = mybir.ActivationFunctionType.Exp

_compiled = {}


def _build():
    nc = bacc.Bacc("TRN2", target_bir_lowering=False, debug=False,
                   num_devices=N_CORES)
    xT_ap = nc.dram_tensor("xT", [D, T], BF16, kind="ExternalInput").ap()
    wq_ap = nc.dram_tensor("wq", [D, HS], BF16, kind="ExternalInput").ap()
    wk_ap = nc.dram_tensor("wk", [D, HS], BF16, kind="ExternalInput").ap()
    wv_ap = nc.dram_tensor("wv", [D, HS], BF16, kind="ExternalInput").ap()
    wo_ap = nc.dram_tensor("wo", [D, HS], BF16, kind="ExternalInput").ap()
    y_ap = nc.dram_tensor("y", [HS, T], F32, kind="ExternalOutput").ap()

    with tile.TileContext(nc) as tc:
        _body(tc, xT_ap, wq_ap, wk_ap, wv_ap, wo_ap, y_ap)
    nc.compile()
    return nc


def _body(tc, xT_ap, wq_ap, wk_ap, wv_ap, wo_ap, y_ap):
    nc = tc.nc
    from contextlib import ExitStack
    with ExitStack() as ctx:
        singles = ctx.enter_context(tc.tile_pool(name="singles", bufs=1))
        work = ctx.enter_context(tc.tile_pool(name="work", bufs=3))
        es_pool = ctx.enter_context(tc.tile_pool(name="es_pool", bufs=6))
        vt_pool = ctx.enter_context(tc.tile_pool(name="vt_pool", bufs=2))
        g_pool = ctx.enter_context(tc.tile_pool(name="g_pool", bufs=2))
        dram = ctx.enter_context(tc.tile_pool(name="dram", bufs=1, space="DRAM"))
        # PSUM budget (8 banks): scores 2x2 + attn@V accum 2 + aux 2
        pa = ctx.enter_context(tc.tile_pool(name="pa", bufs=2, space="PSUM"))
        po = ctx.enter_context(tc.tile_pool(name="po", bufs=1, space="PSUM"))
        paux = ctx.enter_context(tc.tile_pool(name="paux", bufs=2, space="PSUM"))

        # ---- input DMAs, ordered so chunk-0 projections start ASAP ----
        # x chunks split in half-d so the first q-proj matmuls can start
        # after ~0.75MB instead of ~1.25MB.
        wq = singles.tile([128, ND, HS], BF16, tag="wq")
        wk = singles.tile([128, ND, HS], BF16, tag="wk")
        wv = singles.tile([128, ND, HS], BF16, tag="wv")
        wo = singles.tile([128, ND, HS], BF16, tag="wo")
        xcs = [singles.tile([128, ND, QW], BF16, tag=f"x{t}", name=f"xc{t}")
               for t in range(NQ)]
        x_r = xT_ap.rearrange("(c p) (t m) -> p c t m", p=128, t=NQ)
        nc.sync.dma_start(out=wq[:], in_=wq_ap.rearrange("(c p) m -> p c m", p=128))
        nc.sync.dma_start(out=xcs[0][:, 0:4, :], in_=x_r[:, 0:4, 0, :])
        nc.sync.dma_start(out=xcs[0][:, 4:8, :], in_=x_r[:, 4:8, 0, :])
        nc.sync.dma_start(out=wk[:], in_=wk_ap.rearrange("(c p) m -> p c m", p=128))
        nc.sync.dma_start(out=wv[:], in_=wv_ap.rearrange("(c p) m -> p c m", p=128))

        def load_x(t):
            # issued lazily (just before proj(t) is emitted) so early-chunk
            # packets aren't delayed by later chunks round-robining on the
            # same SDMA engines
            nc.sync.dma_start(out=xcs[t][:, 0:4, :], in_=x_r[:, 0:4, t, :])
            nc.sync.dma_start(out=xcs[t][:, 4:8, :], in_=x_r[:, 4:8, t, :])

        ident_f32 = singles.tile([128, 128], F32, tag="ident_f32")
        make_identity(nc, ident_f32)
        ident = singles.tile([128, 128], BF16, tag="ident")
        nc.vector.tensor_copy(ident[:], ident_f32[:])

        qT = singles.tile([128, T], BF16, tag="qT")
        kT = singles.tile([128, T], BF16, tag="kT")
        # V natural layout: [key 128, NK, hd0|1|hd1|1]; head h's stationary
        # operand is vn[:, tk, 65h:65h+65] = [hd, ones] so row 64 of the
        # attn@V output is the softmax denominator.
        vn = singles.tile([128, NK, 2 * (HD + 1)], BF16, tag="vn")
        nc.vector.memset(vn[:, :, HD], 1.0)
        nc.vector.memset(vn[:, :, 2 * HD + 1], 1.0)
        outT = singles.tile([128, T], BF16, tag="outT")

        def proj(t):
            cols = slice(t * QW, (t + 1) * QW)
            for dst, w in ((qT, wq), (kT, wk), (None, wv)):
                ps = paux.tile([128, QW], F32, tag="aux", name=f"pj{t}")
                for d in range(ND):
                    nc.tensor.matmul(ps[:], w[:, d, :], xcs[t][:, d, :],
                                     start=(d == 0), stop=(d == ND - 1))
                if dst is not None:
                    nc.vector.tensor_copy(dst[:, cols], ps[:])
                else:
                    vt = vt_pool.tile([128, QW], BF16, tag="vt", name=f"vt{t}")
                    nc.vector.tensor_copy(vt[:], ps[:])
                    for j in range(4):
                        tk = 4 * t + j
                        ps_t = paux.tile([128, 128], BF16, tag="aux",
                                         name=f"tr{tk}")
                        nc.tensor.transpose(ps_t[:], vt[:, j * W:(j + 1) * W],
                                            ident[:])
                        src = ps_t[:].rearrange("p (h m) -> p h m", h=2)
                        dst3 = vn[:, tk, :].rearrange("p (h m) -> p h m", h=2)
                        nc.vector.tensor_copy(dst3[:, :, 0:HD], src[:])

        # AllGather split 2 chunks + 2 chunks: the first op absorbs the
        # cross-core arrival skew and runs hidden behind chunk-2/3
        # attention; the second starts the moment attention ends.
        ag_in = [dram.tile([HS, 2 * QW], BF16, name=f"ag_in{j}")
                 for j in range(2)]
        ag_out = [dram.tile([N_CORES, HS, 2 * QW], BF16, addr_space="Shared",
                            name=f"ag_out{j}") for j in range(2)]

        def attention(t):
            cols = slice(t * QW, (t + 1) * QW)
            n_tk = 4 * t + 4
            ps_o = [po.tile([HD + 1, QW], F32, tag=f"o{h}", name=f"ps_o{h}")
                    for h in range(HPC)]

            def s_exp(tk):
                qs = max(0, (tk - 4 * t) * W)  # masked cols before qs
                # both heads' scores in one double-wide PSUM tile: the two
                # matmuls use contraction rows 0-63 / 64-127 -> different
                # PE row groups -> concurrent; one ACT instruction exps both
                ps_s = pa.tile([128, 2 * QW], F32, tag="s", name="ps_s")
                for h in range(HPC):
                    hrows = slice(h * HD, (h + 1) * HD)
                    nc.tensor.matmul(
                        ps_s[:, h * QW + qs:(h + 1) * QW],
                        kT[hrows, tk * W:(tk + 1) * W],
                        qT[hrows, t * QW + qs:(t + 1) * QW],
                        start=True, stop=True)
                e = es_pool.tile([128, 2 * QW], BF16, tag="es", name="es")
                nc.scalar.activation(out=e[:, qs:], in_=ps_s[:, qs:],
                                     func=Exp, scale=SCALE)
                return e

            def av(tk, e):
                qs = max(0, (tk - 4 * t) * W)
                for h in range(HPC):
                    nc.tensor.matmul(ps_o[h][:, qs:],
                                     vn[:, tk, h * (HD + 1):(h + 1) * (HD + 1)],
                                     e[:, h * QW + qs:(h + 1) * QW],
                                     start=(tk == 0), stop=(tk == n_tk - 1))

            # scores/exp run two key-chunks ahead of the attn@V accumulation
            pend = [s_exp(0), s_exp(1)]
            for tk in range(2, n_tk):
                pend.append(s_exp(tk))
                av(tk - 2, pend.pop(0))
            av(n_tk - 2, pend.pop(0))
            av(n_tk - 1, pend.pop(0))

            # normalize: rows 0..63 of each ps_o scaled by 1/row64
            for h in range(HPC):
                den = work.tile([1, QW], F32, tag=f"den{h}", name=f"den{t}_{h}")
                nc.vector.tensor_copy(den[:], ps_o[h][HD:HD + 1, :])
                rec = work.tile([1, QW], F32, tag=f"rec{h}", name=f"rec{t}_{h}")
                nc.vector.reciprocal_approx_fast(rec[:], den[:])
                bc = work.tile([HD, QW], F32, tag=f"bc{h}", name=f"bc{t}_{h}")
                nc.gpsimd.partition_broadcast(bc[:], rec[:], channels=HD)
                nc.vector.tensor_mul(outT[h * HD:(h + 1) * HD, cols],
                                     ps_o[h][0:HD, :], bc[:])

            # stage this chunk into its AllGather input buffer
            nc.sync.dma_start(out=ag_in[t // 2][:, (t % 2) * QW:(t % 2 + 1) * QW],
                              in_=outT[:, cols])

        def collect(j):
            nc.gpsimd.collective_compute(
                "AllGather", mybir.AluOpType.bypass,
                replica_groups=[list(range(N_CORES))],
                ins=[ag_in[j].opt()], outs=[ag_out[j].opt()])

        proj(0)
        for t in range(NQ):
            attention(t)
            if t == 1:
                collect(0)
            if t < NQ - 1:
                load_x(t + 1)
                proj(t + 1)

        # Wo only feeds the post-AllGather matmuls (~90us in); issuing its
        # 256KB load here keeps the early DMA lanes clear for x chunks.
        nc.scalar.dma_start(out=wo[:], in_=wo_ap.rearrange("(c p) m -> p c m", p=128))
        collect(1)

        # output projection, per query chunk with pipelined gather reads
        for t in range(NQ):
            cols = slice(t * QW, (t + 1) * QW)
            g_r = ag_out[t // 2].rearrange("c p (u m) -> p c u m", u=2)
            g = g_pool.tile([128, N_CORES, QW], BF16, tag="g", name=f"g{t}")
            nc.scalar.dma_start(out=g[:], in_=g_r[:, :, t % 2, :])
            ps_y = paux.tile([128, QW], F32, tag="aux", name=f"py{t}")
            for c in range(N_CORES):
                nc.tensor.matmul(ps_y[:], wo[:, c, :], g[:, c, :],
                                 start=(c == 0), stop=(c == N_CORES - 1))
            cy = work.tile([128, QW], F32, tag="cy", name=f"cy{t}")
            if t % 2 == 0:
                nc.vector.tensor_copy(cy[:], ps_y[:])
            else:
                nc.scalar.copy(cy[:], ps_y[:])
            nc.sync.dma_start(out=y_ap[:, cols], in_=cy[:])


def _make_in_maps(x, Wq, Wk, Wv, Wo):
    xT = np.asarray(x, np.float32).reshape(T, D).T
    xT = np.ascontiguousarray(xT).astype(bfloat16)
    in_maps = []
    for c in range(N_CORES):
        hs = slice(c * HS, (c + 1) * HS)
        in_maps.append({
            "xT": xT,
            "wq": np.ascontiguousarray(np.asarray(Wq, np.float32)[:, hs]).astype(bfloat16),
            "wk": np.ascontiguousarray(np.asarray(Wk, np.float32)[:, hs]).astype(bfloat16),
            "wv": np.ascontiguousarray(np.asarray(Wv, np.float32)[:, hs]).astype(bfloat16),
            "wo": np.ascontiguousarray(np.asarray(Wo, np.float32)[:, hs]).astype(bfloat16),
        })
    return in_maps


def kernel(x, Wq, Wk, Wv, Wo):
    if "nc" not in _compiled:
        _compiled["nc"] = _build()
    nc = _compiled["nc"]
    in_maps = _make_in_maps(x, Wq, Wk, Wv, Wo)
    res = run_bass_kernel_spmd(nc, in_maps, list(range(N_CORES)))
    finalT = np.concatenate([res.results[c]["y"] for c in range(N_CORES)], axis=0)
    return np.ascontiguousarray(finalT.T).reshape(B, T, D).astype(np.float32)


# revision 18
# speedup vs baseline: 1.2435x; 1.1687x over previous
"""LocalWindowAttention (block-causal) Trainium2 kernel, 8 NeuronCores.

Sharding: tensor-parallel over heads. Core c owns head-columns
[c*128, (c+1)*128) of the D=1024 hidden dim (2 heads x head_dim 64):
  - computes Q/K/V projections for its head slice (transposed layout),
  - block-causal attention for its 2 heads,
  - ONE AllGather of the normalized pre-Wo activations [128, 2048]
    bf16 (the collective runtime here has a ~70us warmup and ~25-40us
    per-op cost, so exactly one op whose input is ready right when
    attention ends is optimal; AllToAll measured WORSE despite moving
    8x less data - per-pair shard overheads dominate),
  - partial output projection with its 128 columns of Wo over the
    gathered activations; core c keeps rows [c*128,(c+1)*128) of
    final^T [1024, 2048]. Host reassembles.

Perf structure (baseline 215us -> v2 173us -> this):
  - bf16 datapath except PSUM accumulation and the final output.
  - fully chunk-pipelined: x streams per 512-column chunk in half-chunk
    DMAs; chunk t+1's projections interleave with chunk t's attention;
    PE starts ~7us in and stays dense (HAM stays at 2.4 GHz).
  - the 2MB full-Wo load is issued AFTER the attention emission so its
    packets don't steal DMA bandwidth from the critical x0 load.
  - reciprocal_approx_fast for softmax denominators (DVE RECIPROCAL is
    ~8 cyc/elem; the approx op is ~5x faster, 18-bit accurate).

Attention runs in S^T layout (keys on partitions, queries free):
S^T tile = K_chunk @ Q^T. No max-subtraction needed (scores bounded).
The two heads' score matmuls use contraction rows 0-63 / 64-127 and
different PSUM banks, so the PE runs them concurrently (row-group
tiling); one ACT instruction computes both heads' exp. V is transposed
to natural layout per chunk with a trailing ones column per head so the
softmax denominator falls out of the attn@V matmul as row 64.
"""

import numpy as np
from ml_dtypes import bfloat16

import concourse.bacc as bacc
import concourse.tile as tile
from concourse import mybir
from concourse.bass_utils import run_bass_kernel_spmd
from concourse.masks import make_identity
from concourse.tile import add_dep_helper

B, T, D = 1, 2048, 1024
H, HD, W = 16, 64, 128
N_CORES = 8
HS = D // N_CORES        # 128 head-columns per core (2 heads)
HPC = H // N_CORES       # heads per core
QW = 512                 # query-chunk width (free dim of S^T tiles)
NQ = T // QW             # 4 query chunks
NK = T // W              # 16 key chunks of 128
ND = D // 128            # 8 contraction chunks over D
SCALE = HD ** -0.5

F32 = mybir.dt.float32
BF16 = mybir.dt.bfloat16
Exp = mybir.ActivationFunctionType.Exp

_compiled = {}


def _build():
    nc = bacc.Bacc("TRN2", target_bir_lowering=False, debug=False,
                   num_devices=N_CORES)
    xT_ap = nc.dram_tensor("xT", [D, T], BF16, kind="ExternalInput").ap()
    wq_ap = nc.dram_tensor("wq", [D, HS], BF16, kind="ExternalInput").ap()
    wk_ap = nc.dram_tensor("wk", [D, HS], BF16, kind="ExternalInput").ap()
    wv_ap = nc.dram_tensor("wv", [D, HS], BF16, kind="ExternalInput").ap()
    wo_ap = nc.dram_tensor("wo", [D, HS], BF16, kind="ExternalInput").ap()
    y_ap = nc.dram_tensor("y", [HS, T], F32, kind="ExternalOutput").ap()

    with tile.TileContext(nc) as tc:
        _body(tc, xT_ap, wq_ap, wk_ap, wv_ap, wo_ap, y_ap)
    nc.compile()
    return nc


def _body(tc, xT_ap, wq_ap, wk_ap, wv_ap, wo_ap, y_ap):
    nc = tc.nc
    from contextlib import ExitStack
    with ExitStack() as ctx:
        singles = ctx.enter_context(tc.tile_pool(name="singles", bufs=1))
        work = ctx.enter_context(tc.tile_pool(name="work", bufs=3))
        es_pool = ctx.enter_context(tc.tile_pool(name="es_pool", bufs=6))
        vt_pool = ctx.enter_context(tc.tile_pool(name="vt_pool", bufs=2))
        g_pool = ctx.enter_context(tc.tile_pool(name="g_pool", bufs=2))
        dram = ctx.enter_context(tc.tile_pool(name="dram", bufs=1, space="DRAM"))
        # PSUM budget (8 banks): scores 2x2 + attn@V accum 2 + aux 2
        pa = ctx.enter_context(tc.tile_pool(name="pa", bufs=2, space="PSUM"))
        po = ctx.enter_context(tc.tile_pool(name="po", bufs=1, space="PSUM"))
        paux = ctx.enter_context(tc.tile_pool(name="paux", bufs=2, space="PSUM"))

        # ---- input DMAs, ordered so chunk-0 projections start ASAP ----
        # x chunks split in half-d so the first q-proj matmuls can start
        # after ~0.75MB instead of ~1.25MB.
        wq = singles.tile([128, ND, HS], BF16, tag="wq")
        wk = singles.tile([128, ND, HS], BF16, tag="wk")
        wv = singles.tile([128, ND, HS], BF16, tag="wv")
        wo = singles.tile([128, ND, HS], BF16, tag="wo")
        xcs = [singles.tile([128, ND, QW], BF16, tag=f"x{t}", name=f"xc{t}")
               for t in range(NQ)]
        x_r = xT_ap.rearrange("(c p) (t m) -> p c t m", p=128, t=NQ)
        nc.sync.dma_start(out=wq[:], in_=wq_ap.rearrange("(c p) m -> p c m", p=128))
        nc.sync.dma_start(out=xcs[0][:, 0:4, :], in_=x_r[:, 0:4, 0, :])
        nc.sync.dma_start(out=xcs[0][:, 4:8, :], in_=x_r[:, 4:8, 0, :])
        nc.sync.dma_start(out=wk[:], in_=wk_ap.rearrange("(c p) m -> p c m", p=128))
        nc.sync.dma_start(out=wv[:], in_=wv_ap.rearrange("(c p) m -> p c m", p=128))

        # tiny dummy AllGather issued immediately: absorbs the cross-core
        # start skew and ncfw warmup concurrently with the compute so the
        # real collectives later start promptly
        dum_in = dram.tile([1, 64], BF16, name="dum_in")
        dum_out = dram.tile([N_CORES, 1, 64], BF16, addr_space="Shared",
                            name="dum_out")
        nc.gpsimd.collective_compute(
            "AllGather", mybir.AluOpType.bypass,
            replica_groups=[list(range(N_CORES))],
            ins=[dum_in.opt()], outs=[dum_out.opt()])

        def load_x(t):
            # issued lazily (just before proj(t) is emitted) so early-chunk
            # packets aren't delayed by later chunks round-robining on the
            # same SDMA engines
            nc.sync.dma_start(out=xcs[t][:, 0:4, :], in_=x_r[:, 0:4, t, :])
            nc.sync.dma_start(out=xcs[t][:, 4:8, :], in_=x_r[:, 4:8, t, :])

        ident_f32 = singles.tile([128, 128], F32, tag="ident_f32")
        make_identity(nc, ident_f32)
        ident = singles.tile([128, 128], BF16, tag="ident")
        nc.vector.tensor_copy(ident[:], ident_f32[:])

        qT = singles.tile([128, T], BF16, tag="qT")
        kT = singles.tile([128, T], BF16, tag="kT")
        # V natural layout: [key 128, NK, hd0|1|hd1|1]; head h's stationary
        # operand is vn[:, tk, 65h:65h+65] = [hd, ones] so row 64 of the
        # attn@V output is the softmax denominator.
        vn = singles.tile([128, NK, 2 * (HD + 1)], BF16, tag="vn")
        nc.vector.memset(vn[:, :, HD], 1.0)
        nc.vector.memset(vn[:, :, 2 * HD + 1], 1.0)
        outT = singles.tile([128, T], BF16, tag="outT")

        def proj(t):
            cols = slice(t * QW, (t + 1) * QW)
            for dst, w in ((qT, wq), (kT, wk), (None, wv)):
                ps = paux.tile([128, QW], F32, tag="aux", name=f"pj{t}")
                for d in range(ND):
                    nc.tensor.matmul(ps[:], w[:, d, :], xcs[t][:, d, :],
                                     start=(d == 0), stop=(d == ND - 1))
                if dst is not None:
                    nc.vector.tensor_copy(dst[:, cols], ps[:])
                else:
                    vt = vt_pool.tile([128, QW], BF16, tag="vt", name=f"vt{t}")
                    nc.vector.tensor_copy(vt[:], ps[:])
                    for j in range(4):
                        tk = 4 * t + j
                        ps_t = paux.tile([128, 128], BF16, tag="aux",
                                         name=f"tr{tk}")
                        nc.tensor.transpose(ps_t[:], vt[:, j * W:(j + 1) * W],
                                            ident[:])
                        src = ps_t[:].rearrange("p (h m) -> p h m", h=2)
                        dst3 = vn[:, tk, :].rearrange("p (h m) -> p h m", h=2)
                        nc.vector.tensor_copy(dst3[:, :, 0:HD], src[:])

        # AllGather split 2 chunks + 2 chunks: the first op absorbs the
        # cross-core arrival skew and runs hidden behind chunk-2/3
        # attention; the second starts the moment attention ends.
        ag_in = [dram.tile([HS, 2 * QW], BF16, name=f"ag_in{j}")
                 for j in range(2)]
        ag_out = [dram.tile([N_CORES, HS, 2 * QW], BF16, addr_space="Shared",
                            name=f"ag_out{j}") for j in range(2)]

        def attention(t):
            cols = slice(t * QW, (t + 1) * QW)
            n_tk = 4 * t + 4
            ps_o = [po.tile([HD + 1, QW], F32, tag=f"o{h}", name=f"ps_o{h}")
                    for h in range(HPC)]

            def s_exp(tk):
                qs = max(0, (tk - 4 * t) * W)  # masked cols before qs
                # both heads' scores in one double-wide PSUM tile: the two
                # matmuls use contraction rows 0-63 / 64-127 -> different
                # PE row groups -> concurrent; one ACT instruction exps both
                ps_s = pa.tile([128, 2 * QW], F32, tag="s", name="ps_s")
                for h in range(HPC):
                    hrows = slice(h * HD, (h + 1) * HD)
                    nc.tensor.matmul(
                        ps_s[:, h * QW + qs:(h + 1) * QW],
                        kT[hrows, tk * W:(tk + 1) * W],
                        qT[hrows, t * QW + qs:(t + 1) * QW],
                        start=True, stop=True)
                e = es_pool.tile([128, 2 * QW], BF16, tag="es", name="es")
                nc.scalar.activation(out=e[:, qs:], in_=ps_s[:, qs:],
                                     func=Exp, scale=SCALE)
                return e

            def av(tk, e):
                qs = max(0, (tk - 4 * t) * W)
                for h in range(HPC):
                    nc.tensor.matmul(ps_o[h][:, qs:],
                                     vn[:, tk, h * (HD + 1):(h + 1) * (HD + 1)],
                                     e[:, h * QW + qs:(h + 1) * QW],
                                     start=(tk == 0), stop=(tk == n_tk - 1))

            # scores/exp run two key-chunks ahead of the attn@V accumulation
            pend = [s_exp(0), s_exp(1)]
            for tk in range(2, n_tk):
                pend.append(s_exp(tk))
                av(tk - 2, pend.pop(0))
            av(n_tk - 2, pend.pop(0))
            av(n_tk - 1, pend.pop(0))

            # normalize: rows 0..63 of each ps_o scaled by 1/row64
            for h in range(HPC):
                den = work.tile([1, QW], F32, tag=f"den{h}", name=f"den{t}_{h}")
                nc.vector.tensor_copy(den[:], ps_o[h][HD:HD + 1, :])
                rec = work.tile([1, QW], F32, tag=f"rec{h}", name=f"rec{t}_{h}")
                nc.vector.reciprocal_approx_fast(rec[:], den[:])
                bc = work.tile([HD, QW], F32, tag=f"bc{h}", name=f"bc{t}_{h}")
                nc.gpsimd.partition_broadcast(bc[:], rec[:], channels=HD)
                nc.vector.tensor_mul(outT[h * HD:(h + 1) * HD, cols],
                                     ps_o[h][0:HD, :], bc[:])

            # stage this chunk into its AllGather input buffer
            nc.sync.dma_start(out=ag_in[t // 2][:, (t % 2) * QW:(t % 2 + 1) * QW],
                              in_=outT[:, cols])

        coll = [None, None]

        def collect(j):
            coll[j] = nc.gpsimd.collective_compute(
                "AllGather", mybir.AluOpType.bypass,
                replica_groups=[list(range(N_CORES))],
                ins=[ag_in[j].opt()], outs=[ag_out[j].opt()])

        proj(0)
        for t in range(NQ):
            if t < NQ - 1:
                load_x(t + 1)
            attention(t)
            if t == 1:
                collect(0)
            if t < NQ - 1:
                proj(t + 1)

        # Wo only feeds the post-AllGather matmuls (~90us in); issuing its
        # 256KB load here keeps the early DMA lanes clear for x chunks.
        nc.scalar.dma_start(out=wo[:], in_=wo_ap.rearrange("(c p) m -> p c m", p=128))
        collect(1)

        # output projection, per query chunk with pipelined gather reads
        for t in range(NQ):
            cols = slice(t * QW, (t + 1) * QW)
            g_r = ag_out[t // 2].rearrange("c p (u m) -> p c u m", u=2)
            g = g_pool.tile([128, N_CORES, QW], BF16, tag="g", name=f"g{t}")
            gi = nc.scalar.dma_start(out=g[:], in_=g_r[:, :, t % 2, :])
            # keep the gather reads behind the final AG trigger in the
            # scalar stream: the scheduler's cost model treats collectives
            # as fast, would hoist these above chunk-3's exps, and the
            # strict-FIFO ACT queue then head-of-line blocks on AG-done
            add_dep_helper(gi.ins, coll[1].ins, sync=False,
                           reason="g load scheduled after final AG trigger")
            ps_y = paux.tile([128, QW], F32, tag="aux", name=f"py{t}")
            for c in range(N_CORES):
                nc.tensor.matmul(ps_y[:], wo[:, c, :], g[:, c, :],
                                 start=(c == 0), stop=(c == N_CORES - 1))
            cy = work.tile([128, QW], F32, tag="cy", name=f"cy{t}")
            if t % 2 == 0:
                nc.vector.tensor_copy(cy[:], ps_y[:])
            else:
                nc.scalar.copy(cy[:], ps_y[:])
            nc.sync.dma_start(out=y_ap[:, cols], in_=cy[:])


def _make_in_maps(x, Wq, Wk, Wv, Wo):
    xT = np.asarray(x, np.float32).reshape(T, D).T
    xT = np.ascontiguousarray(xT).astype(bfloat16)
    in_maps = []
    for c in range(N_CORES):
        hs = slice(c * HS, (c + 1) * HS)
        in_maps.append({
            "xT": xT,
            "wq": np.ascontiguousarray(np.asarray(Wq, np.float32)[:, hs]).astype(bfloat16),
            "wk": np.ascontiguousarray(np.asarray(Wk, np.float32)[:, hs]).astype(bfloat16),
            "wv": np.ascontiguousarray(np.asarray(Wv, np.float32)[:, hs]).astype(bfloat16),
            "wo": np.ascontiguousarray(np.asarray(Wo, np.float32)[:, hs]).astype(bfloat16),
        })
    return in_maps


def kernel(x, Wq, Wk, Wv, Wo):
    if "nc" not in _compiled:
        _compiled["nc"] = _build()
    nc = _compiled["nc"]
    in_maps = _make_in_maps(x, Wq, Wk, Wv, Wo)
    res = run_bass_kernel_spmd(nc, in_maps, list(range(N_CORES)))
    finalT = np.concatenate([res.results[c]["y"] for c in range(N_CORES)], axis=0)
    return np.ascontiguousarray(finalT.T).reshape(B, T, D).astype(np.float32)
